# revision 12
# baseline (speedup 1.0000x reference)
"""Trainium2 Bass kernel for differentiable KDE (Gaussian kernel density).

Math (h = 1, C = 0.5/sqrt(2*pi)):
    density[i] = mean_j exp(-C * ||x_i - d_j||^2)
               = sum_j exp(2C x_i.d_j - C||d_j||^2 - C||x_i||^2 - lnM)

Sharding: data-parallel over x rows (1024 per core), data replicated.

Per-core architecture (i = x row as PSUM partition, j = data row as free dim):
    - ACT (scalar) is the hard floor: 8.39M exps at 1 elem/cycle/lane
      @1.2GHz ~= 55us. Everything else is kept below it and overlapped.
    - Data path: DMA f32 rows -> DVE cast bf16 -> DMA to DRAM scratch ->
      DMA-xbar-transpose back as dataT [128(d), M] bf16 (no PSUM, no PE).
      Norms come from the same bf16 values, so the -C||d_j||^2 term
      matches the quantized data exactly; same for x (bf16 + matching
      norm bias), making each kernel term exact for the rounded points.
    - Main loop, per (j-superblock of 2048) x (i-block of 128):
      PE: 4x matmul psum[128, 512] = xT_b.T @ dataT (bf16) plus a rank-1
      fp16 matmul (ones x dnrow) accumulating -C||d_j||^2 into psum;
      ACT: e = exp(2C*psum + bias_i) -> bf16 (bias = -C||x_i||^2 - lnM);
      DVE: tensor_scalar (mult 1, add 0) with accum_out -> per-partition
      running sums at 4x DVE rate.
    - PSUM: 2 x [128, 2048] main tiles = all 8 banks, double-buffered.
"""
import math
from contextlib import ExitStack

import numpy as np

from concourse import bacc, mybir, tile
from concourse.bass_utils import run_bass_kernel_spmd
from concourse import masks

N, M, D = 8192, 8192, 128
NCORES = 8
NS = N // NCORES            # 1024 x-rows per core
P = 128                     # partitions
NB = NS // P                # 8 i-blocks
NCH = 8                     # data chunks (1024 rows each)
RPC = M // NCH              # 1024 rows per chunk
RPP = RPC // P              # 8 rows per partition per chunk
NJS = 4                     # j-superblocks
JW = M // NJS               # 2048 j per superblock

C = 0.5 / math.sqrt(2.0 * math.pi)
TWO_C = 2.0 * C
LNM = math.log(float(M))

F32 = mybir.dt.float32
BF16 = mybir.dt.bfloat16
FP16 = mybir.dt.float16

_CACHED_NC = None


def _build():
    nc = bacc.Bacc("TRN2", target_bir_lowering=False, debug=False)
    x_d = nc.dram_tensor("x", [NS, D], F32, kind="ExternalInput")
    d_d = nc.dram_tensor("data", [M, D], F32, kind="ExternalInput")
    o_d = nc.dram_tensor("out", [NS, 1], F32, kind="ExternalOutput")
    dbf_d = nc.dram_tensor("dbf", [M, D], BF16, kind="Internal")
    nscr_d = nc.dram_tensor("nscr", [M], FP16, kind="Internal")

    # row p*RPP + r lands at [p, r, :]: 4KB contiguous per partition
    x_re = x_d.ap().rearrange("(p r) d -> p r d", p=P)          # [128, 8, 128]
    d_re = d_d.ap().rearrange("(c p r) d -> c p r d", c=NCH, p=P)
    dbf_re = dbf_d.ap().rearrange("(c p r) d -> c p r d", c=NCH, p=P)

    with tile.TileContext(nc) as tc, ExitStack() as ctx:
        const_pool = ctx.enter_context(tc.tile_pool(name="const", bufs=1))
        big_pool = ctx.enter_context(tc.tile_pool(name="big", bufs=1))
        drow_pool = ctx.enter_context(tc.tile_pool(name="drow", bufs=3))
        dbf_pool = ctx.enter_context(tc.tile_pool(name="dbf", bufs=2))
        gsq_pool = ctx.enter_context(tc.tile_pool(name="gsq", bufs=2))
        e_pool = ctx.enter_context(tc.tile_pool(name="e", bufs=3))
        ep_pool = ctx.enter_context(tc.tile_pool(name="ep", bufs=2))
        ps_main = ctx.enter_context(tc.tile_pool(name="psm", bufs=2, space="PSUM"))

        ident = const_pool.tile([P, P], F32, tag="ident")
        masks.make_identity(nc, ident[:])
        ones16 = const_pool.tile([1, P], FP16, tag="ones16")
        nc.gpsimd.memset(ones16[:], 1.0)

        xrow = big_pool.tile([P, NB, P], F32, tag="xrow")
        xbf = big_pool.tile([P, NB, P], BF16, tag="xbf")
        xT = big_pool.tile([P, NS], BF16, tag="xT")
        xsqs = big_pool.tile([P, NB * P], F32, tag="xsqs")
        xnsq = big_pool.tile([P, NB], F32, tag="xnsq")
        xbias = big_pool.tile([P, NB], F32, tag="xbias")
        dataT = big_pool.tile([P, M], BF16, tag="dataT")
        dnsq = big_pool.tile([P, NCH * RPP], F32, tag="dnsq")
        dnst = big_pool.tile([P, RPP], FP16, tag="dnst")
        dnrow = big_pool.tile([1, M], FP16, tag="dnrow")
        pacc = big_pool.tile([P, NB * NJS], F32, tag="pacc")
        outsb = big_pool.tile([P, NB], F32, tag="outsb")

        # ---- x prologue ----
        nc.sync.dma_start(xrow[:], x_re)
        # bf16-quantize x; norms use the same quantized values as the matmul
        nc.vector.tensor_copy(xbf[:], xrow[:])
        nc.vector.tensor_mul(
            xsqs[:], xbf[:].rearrange("p a b -> p (a b)"),
            xbf[:].rearrange("p a b -> p (a b)"))
        nc.vector.tensor_reduce(
            xnsq[:].rearrange("p (r o) -> p r o", o=1),
            xsqs[:].rearrange("p (r d) -> p r d", d=P),
            axis=mybir.AxisListType.X, op=mybir.AluOpType.add)
        nc.gpsimd.tensor_scalar_mul(xbias[:], xnsq[:], -C)
        nc.gpsimd.tensor_scalar_add(xbias[:], xbias[:], -LNM)
        # x transposes through one main psum tile (before main loop needs it)
        pmx = ps_main.tile([P, JW], F32, tag="pm")
        for t in range(NB):
            nc.tensor.transpose(pmx[:, t * P:(t + 1) * P], xrow[:, t, :],
                                ident[:])
        nc.vector.tensor_copy(xT[:], pmx[:, 0:NS])

        # ---- streamed data prologue + main loop ----
        for ch in range(NCH):
            rsl = slice(ch * RPC, (ch + 1) * RPC)
            drow = drow_pool.tile([P, RPP, P], F32, tag="drow")
            nc.sync.dma_start(drow[:], d_re[ch])
            dbf = dbf_pool.tile([P, RPP, P], BF16, tag="dbf")
            nc.vector.tensor_copy(dbf[:], drow[:])
            # bf16 rows -> DRAM -> xbar-transposed back (same queue = ordered)
            nc.scalar.dma_start(dbf_re[ch], dbf[:])
            nc.scalar.dma_start_transpose(dataT[:, rsl], dbf_d.ap()[rsl, :])
            # norms from the same bf16 values the matmul will see
            g = gsq_pool.tile([P, RPC], F32, tag="gsq")
            dbf_f = dbf[:].rearrange("p a b -> p (a b)")
            nc.gpsimd.tensor_mul(g[:], dbf_f, dbf_f)
            csl = slice(ch * RPP, (ch + 1) * RPP)
            nc.vector.tensor_reduce(
                dnsq[:, csl].rearrange("p (r o) -> p r o", o=1),
                g[:].rearrange("p (r d) -> p r d", d=P),
                axis=mybir.AxisListType.X, op=mybir.AluOpType.add)
            # -||d_j||^2/2 as fp16 row in j order (ACT applies the 2C scale
            # to the whole psum, giving -C||d_j||^2): p-major flat via DRAM
            nc.gpsimd.tensor_scalar_mul(dnst[:], dnsq[:, csl], -0.5)
            nc.sync.dma_start(
                nscr_d.ap()[rsl].rearrange("(p r) -> p r", p=P), dnst[:])
            nc.sync.dma_start(
                dnrow[:, rsl],
                nscr_d.ap()[rsl].rearrange("(o q) -> o q", o=1))

            if ch % 2 == 1:
                js = ch // 2
                for b in range(NB):
                    pm = ps_main.tile([P, JW], F32, tag="pm")
                    lhs = xT[:, b * P:(b + 1) * P]
                    for q in range(4):
                        qsl = slice(q * 512, (q + 1) * 512)
                        jsl = slice(js * JW + q * 512, js * JW + (q + 1) * 512)
                        nc.tensor.matmul(pm[:, qsl], lhs, dataT[:, jsl],
                                         start=True, stop=False)
                        nc.tensor.matmul(pm[:, qsl], ones16[:], dnrow[:, jsl],
                                         start=False, stop=True)
                    e = e_pool.tile([P, JW], BF16, tag="e")
                    nc.scalar.activation(e[:], pm[:],
                                         mybir.ActivationFunctionType.Exp,
                                         bias=xbias[:, b:b + 1], scale=TWO_C)
                    ep = ep_pool.tile([P, JW], BF16, tag="ep")
                    nc.vector.tensor_scalar(
                        out=ep[:], in0=e[:], scalar1=1.0, scalar2=0.0,
                        op0=mybir.AluOpType.mult, op1=mybir.AluOpType.add,
                        accum_out=pacc[:, b * NJS + js:b * NJS + js + 1])

        # ---- epilogue: density row p*8+b at outsb[p, b] ----
        for b in range(NB):
            nc.vector.tensor_reduce(
                outsb[:, b:b + 1], pacc[:, b * NJS:(b + 1) * NJS],
                axis=mybir.AxisListType.X, op=mybir.AluOpType.add)
        nc.sync.dma_start(
            o_d.ap().rearrange("(p r) o -> p (r o)", p=P), outsb[:])

    nc.compile()
    return nc


def kernel(x, data):
    global _CACHED_NC
    x = np.ascontiguousarray(np.asarray(x, dtype=np.float32))
    data = np.ascontiguousarray(np.asarray(data, dtype=np.float32))
    assert x.shape == (N, D) and data.shape == (M, D)

    if _CACHED_NC is None:
        _CACHED_NC = _build()
    nc = _CACHED_NC

    in_maps = [
        {"x": x[c * NS:(c + 1) * NS], "data": data} for c in range(NCORES)
    ]
    res = run_bass_kernel_spmd(nc, in_maps, list(range(NCORES)))
    dens = np.concatenate(
        [np.asarray(res.results[c]["out"]).reshape(NS) for c in range(NCORES)]
    )
    return dens.reshape(N, 1).astype(np.float32)


if __name__ == "__main__":
    rng = np.random.default_rng(0)
    x = rng.standard_normal((N, D), dtype=np.float32)
    data = rng.standard_normal((M, D), dtype=np.float32)
    out = kernel(x, data)
    print("kernel out", out.shape, out[:4, 0])


# revision 15
# speedup vs baseline: 1.1030x; 1.1030x over previous
"""Trainium2 Bass kernel for differentiable KDE (Gaussian kernel density).

Math (h = 1, C = 0.5/sqrt(2*pi)):
    density[i] = mean_j exp(-C * ||x_i - d_j||^2)
               = sum_j exp(2C x_i.d_j - C||d_j||^2 - C||x_i||^2 - lnM)

Sharding: data-parallel over x rows (1024 per core), data replicated.

Per-core architecture (i = x row as PSUM partition, j = data row as free dim):
    - ACT (scalar) is the hard floor: 8.39M exps at 1 elem/cycle/lane
      @1.2GHz ~= 55us. Everything else is kept below it and overlapped.
    - Data path: DMA f32 rows -> DVE cast bf16 -> DMA to DRAM scratch ->
      DMA-xbar-transpose back as dataT [128(d), M] bf16 (no PSUM, no PE).
      Norms come from the same bf16 values, so the -C||d_j||^2 term
      matches the quantized data exactly; same for x (bf16 + matching
      norm bias), making each kernel term exact for the rounded points.
    - Main loop, per (j-superblock of 2048) x (i-block of 128):
      PE: 4x matmul psum[128, 512] = xT_b.T @ dataT (bf16) plus a rank-1
      fp16 matmul (ones x dnrow) accumulating -C||d_j||^2 into psum;
      ACT: e = exp(2C*psum + bias_i) -> bf16 (bias = -C||x_i||^2 - lnM);
      DVE: tensor_scalar (mult 1, add 0) with accum_out -> per-partition
      running sums at 4x DVE rate.
    - PSUM: 2 x [128, 2048] main tiles = all 8 banks, double-buffered.
"""
import math
from contextlib import ExitStack

import numpy as np

from concourse import bacc, mybir, tile
from concourse.bass_utils import run_bass_kernel_spmd
from concourse import masks

N, M, D = 8192, 8192, 128
NCORES = 8
NS = N // NCORES            # 1024 x-rows per core
P = 128                     # partitions
NB = NS // P                # 8 i-blocks
NCH = 8                     # data chunks (1024 rows each)
RPC = M // NCH              # 1024 rows per chunk
RPP = RPC // P              # 8 rows per partition per chunk
NJS = 4                     # j-superblocks
JW = M // NJS               # 2048 j per superblock

C = 0.5 / math.sqrt(2.0 * math.pi)
TWO_C = 2.0 * C
LNM = math.log(float(M))

F32 = mybir.dt.float32
BF16 = mybir.dt.bfloat16
FP16 = mybir.dt.float16

_CACHED_NC = None


def _build():
    nc = bacc.Bacc("TRN2", target_bir_lowering=False, debug=False)
    x_d = nc.dram_tensor("x", [NS, D], F32, kind="ExternalInput")
    d_d = nc.dram_tensor("data", [M, D], F32, kind="ExternalInput")
    o_d = nc.dram_tensor("out", [NS, 1], F32, kind="ExternalOutput")
    dbf_d = nc.dram_tensor("dbf", [M, D], BF16, kind="Internal")
    nscr_d = nc.dram_tensor("nscr", [M], FP16, kind="Internal")

    # row p*RPP + r lands at [p, r, :]: 4KB contiguous per partition
    x_re = x_d.ap().rearrange("(p r) d -> p r d", p=P)          # [128, 8, 128]
    d_re = d_d.ap().rearrange("(c p r) d -> c p r d", c=NCH, p=P)
    dbf_re = dbf_d.ap().rearrange("(c p r) d -> c p r d", c=NCH, p=P)

    with tile.TileContext(nc) as tc, ExitStack() as ctx:
        const_pool = ctx.enter_context(tc.tile_pool(name="const", bufs=1))
        big_pool = ctx.enter_context(tc.tile_pool(name="big", bufs=1))
        drow_pool = ctx.enter_context(tc.tile_pool(name="drow", bufs=3))
        dbf_pool = ctx.enter_context(tc.tile_pool(name="dbf", bufs=2))
        gsq_pool = ctx.enter_context(tc.tile_pool(name="gsq", bufs=2))
        ps_main = ctx.enter_context(tc.tile_pool(name="psm", bufs=2, space="PSUM"))

        ident = const_pool.tile([P, P], F32, tag="ident")
        masks.make_identity(nc, ident[:])
        ones16 = const_pool.tile([1, P], FP16, tag="ones16")
        nc.gpsimd.memset(ones16[:], 1.0)

        xrow = big_pool.tile([P, NB, P], F32, tag="xrow")
        xbf = big_pool.tile([P, NB, P], BF16, tag="xbf")
        xT = big_pool.tile([P, NS], BF16, tag="xT")
        xsqs = big_pool.tile([P, NB * P], F32, tag="xsqs")
        xnsq = big_pool.tile([P, NB], F32, tag="xnsq")
        xbias = big_pool.tile([P, NB], F32, tag="xbias")
        dataT = big_pool.tile([P, M], BF16, tag="dataT")
        dnsq = big_pool.tile([P, NCH * RPP], F32, tag="dnsq")
        dnst = big_pool.tile([P, RPP], FP16, tag="dnst")
        dnrow = big_pool.tile([1, M], FP16, tag="dnrow")
        pacc = big_pool.tile([P, NB * NJS], F32, tag="pacc")
        outsb = big_pool.tile([P, NB], F32, tag="outsb")
        # e is write-only scratch (ACT's accumulator carries the reduction)
        escr = big_pool.tile([P, JW], BF16, tag="escr")

        # ---- x prologue ----
        nc.sync.dma_start(xrow[:], x_re)
        # bf16-quantize x; norms use the same quantized values as the matmul
        nc.vector.tensor_copy(xbf[:], xrow[:])
        nc.vector.tensor_mul(
            xsqs[:], xbf[:].rearrange("p a b -> p (a b)"),
            xbf[:].rearrange("p a b -> p (a b)"))
        nc.vector.tensor_reduce(
            xnsq[:].rearrange("p (r o) -> p r o", o=1),
            xsqs[:].rearrange("p (r d) -> p r d", d=P),
            axis=mybir.AxisListType.X, op=mybir.AluOpType.add)
        nc.gpsimd.tensor_scalar_mul(xbias[:], xnsq[:], -C)
        nc.gpsimd.tensor_scalar_add(xbias[:], xbias[:], -LNM)
        # x transposes through one main psum tile (before main loop needs it)
        pmx = ps_main.tile([P, JW], F32, tag="pm")
        for t in range(NB):
            nc.tensor.transpose(pmx[:, t * P:(t + 1) * P], xrow[:, t, :],
                                ident[:])
        nc.vector.tensor_copy(xT[:], pmx[:, 0:NS])

        # ---- streamed data prologue + main loop ----
        for ch in range(NCH):
            rsl = slice(ch * RPC, (ch + 1) * RPC)
            drow = drow_pool.tile([P, RPP, P], F32, tag="drow")
            nc.sync.dma_start(drow[:], d_re[ch])
            dbf = dbf_pool.tile([P, RPP, P], BF16, tag="dbf")
            nc.vector.tensor_copy(dbf[:], drow[:])
            # bf16 rows -> DRAM -> xbar-transposed back (same queue = ordered)
            nc.scalar.dma_start(dbf_re[ch], dbf[:])
            nc.scalar.dma_start_transpose(dataT[:, rsl], dbf_d.ap()[rsl, :])
            # norms from the same bf16 values the matmul will see
            g = gsq_pool.tile([P, RPC], F32, tag="gsq")
            dbf_f = dbf[:].rearrange("p a b -> p (a b)")
            nc.gpsimd.tensor_mul(g[:], dbf_f, dbf_f)
            csl = slice(ch * RPP, (ch + 1) * RPP)
            nc.vector.tensor_reduce(
                dnsq[:, csl].rearrange("p (r o) -> p r o", o=1),
                g[:].rearrange("p (r d) -> p r d", d=P),
                axis=mybir.AxisListType.X, op=mybir.AluOpType.add)
            # -||d_j||^2/2 as fp16 row in j order (ACT applies the 2C scale
            # to the whole psum, giving -C||d_j||^2): p-major flat via DRAM
            nc.gpsimd.tensor_scalar_mul(dnst[:], dnsq[:, csl], -0.5)
            nc.sync.dma_start(
                nscr_d.ap()[rsl].rearrange("(p r) -> p r", p=P), dnst[:])
            nc.sync.dma_start(
                dnrow[:, rsl],
                nscr_d.ap()[rsl].rearrange("(o q) -> o q", o=1))

            if ch % 2 == 1:
                js = ch // 2
                for b in range(NB):
                    pm = ps_main.tile([P, JW], F32, tag="pm")
                    lhs = xT[:, b * P:(b + 1) * P]
                    # grouped by stationary operand: 4x rank-1 bias, then
                    # 4x main, so weights swap twice per tile, not 8 times
                    for q in range(4):
                        qsl = slice(q * 512, (q + 1) * 512)
                        jsl = slice(js * JW + q * 512, js * JW + (q + 1) * 512)
                        nc.tensor.matmul(pm[:, qsl], ones16[:], dnrow[:, jsl],
                                         start=True, stop=False)
                    for q in range(4):
                        qsl = slice(q * 512, (q + 1) * 512)
                        jsl = slice(js * JW + q * 512, js * JW + (q + 1) * 512)
                        nc.tensor.matmul(pm[:, qsl], lhs, dataT[:, jsl],
                                         start=False, stop=True)
                    nc.scalar.activation(
                        escr[:], pm[:], mybir.ActivationFunctionType.Exp,
                        bias=xbias[:, b:b + 1], scale=TWO_C,
                        accum_out=pacc[:, b * NJS + js:b * NJS + js + 1])

        # ---- epilogue: density row p*8+b at outsb[p, b] ----
        for b in range(NB):
            nc.vector.tensor_reduce(
                outsb[:, b:b + 1], pacc[:, b * NJS:(b + 1) * NJS],
                axis=mybir.AxisListType.X, op=mybir.AluOpType.add)
        nc.sync.dma_start(
            o_d.ap().rearrange("(p r) o -> p (r o)", p=P), outsb[:])

    nc.compile()
    return nc


def kernel(x, data):
    global _CACHED_NC
    x = np.ascontiguousarray(np.asarray(x, dtype=np.float32))
    data = np.ascontiguousarray(np.asarray(data, dtype=np.float32))
    assert x.shape == (N, D) and data.shape == (M, D)

    if _CACHED_NC is None:
        _CACHED_NC = _build()
    nc = _CACHED_NC

    in_maps = [
        {"x": x[c * NS:(c + 1) * NS], "data": data} for c in range(NCORES)
    ]
    res = run_bass_kernel_spmd(nc, in_maps, list(range(NCORES)))
    dens = np.concatenate(
        [np.asarray(res.results[c]["out"]).reshape(NS) for c in range(NCORES)]
    )
    return dens.reshape(N, 1).astype(np.float32)


if __name__ == "__main__":
    rng = np.random.default_rng(0)
    x = rng.standard_normal((N, D), dtype=np.float32)
    data = rng.standard_normal((M, D), dtype=np.float32)
    out = kernel(x, data)
    print("kernel out", out.shape, out[:4, 0])
